# revision 25
# baseline (speedup 1.0000x reference)
"""Trainium2 8-core tensor-parallel attention kernel (Bass/Tile), v2.

Full inputs in, full output out. Sharding: tensor-parallel over heads
(4 heads per core), per-batch AllGather of attention outputs overlapped
with compute, each core computes a 512-wide output-column slice of the
o_proj; host concatenates.

v2 vs baseline:
- RoPE via evens/odds head-dim permutation (folded into wq/wk host-side):
  the pair rotation becomes two contiguous 64-partition SBUF-SBUF DMA
  copies + sign-folded sin, removing the rotation matmul and a PSUM bank.
- Causal trim: score/PV/sum matmuls skip fully-masked 128-column blocks.
- Mask via affine_select on the exp output (no mask tensor at all).
- Row-sums batched into a shared PSUM tile (32-row replicated ones), one
  fast reciprocal per jq-pair.
- PSUM/SBUF pools nested so attention overlaps phase 1 (b1) and o_proj
  (b0) overlaps attention (b1); AllGathers hide under compute.
"""
import sys

for _p in ("/opt/trn_rl_repo",):
    if _p not in sys.path:
        sys.path.insert(0, _p)

import numpy as np
import ml_dtypes

import concourse.bass as bass
import concourse.mybir as mybir
import concourse.tile as tile
from concourse import bacc
from concourse.bass_utils import run_bass_kernel_spmd

B, S, D, H = 2, 2048, 4096, 32
HD = D // H          # 128 head dim
T = B * S            # 4096 tokens
NC = 8               # cores
HL = H // NC         # 4 heads per core
DH = HL * HD         # 512 dims per core
SCALE = 1.0 / float(np.sqrt(HD))
BF16 = mybir.dt.bfloat16
F32 = mybir.dt.float32
bf16 = ml_dtypes.bfloat16

_CACHE = {}
LAST_RESULT = None


def build():
    nc = bacc.Bacc("TRN2", target_bir_lowering=False, debug=False, num_devices=NC)

    xT = nc.dram_tensor("xT", [D, T], BF16, kind="ExternalInput").ap()
    wqT = nc.dram_tensor("wqT", [D, DH], BF16, kind="ExternalInput").ap()
    wkT = nc.dram_tensor("wkT", [D, DH], BF16, kind="ExternalInput").ap()
    wvT = nc.dram_tensor("wvT", [D, DH], BF16, kind="ExternalInput").ap()
    woT = nc.dram_tensor("woT", [D, DH], BF16, kind="ExternalInput").ap()
    cos2E = nc.dram_tensor("cos2E", [HD, T], BF16, kind="ExternalInput").ap()
    sin2E = nc.dram_tensor("sin2E", [HD, T], BF16, kind="ExternalInput").ap()
    ones_k = nc.dram_tensor("ones_k", [128, 32], BF16, kind="ExternalInput").ap()
    ones_b = nc.dram_tensor("ones_b", [128, 128], BF16, kind="ExternalInput").ap()
    mask128 = nc.dram_tensor("mask128", [128, 128], F32, kind="ExternalInput").ap()
    out = nc.dram_tensor("out", [T, DH], F32, kind="ExternalOutput").ap()

    NT = T // 512      # 8 token slices of 512
    NCT = D // 128     # 32 contraction tiles

    with tile.TileContext(nc) as tc:
        with tc.tile_pool(name="dram", bufs=1, space="DRAM") as dram:
            qTd = dram.tile([DH, T], BF16)
            kTd = dram.tile([DH, T], BF16)
            vd = dram.tile([T, DH], BF16)
            agin = {(b, h): dram.tile([128, S], BF16, name=f"agin{b}{h}")
                    for b in range(B) for h in range(HL)}
            agout = {(b, h): dram.tile([NC * 128, S], BF16, addr_space="Shared",
                                       name=f"agout{b}{h}")
                     for b in range(B) for h in range(HL)}

            # Outer pools spanning attention + o_proj (opened first so the
            # phase-1 pools below release their space LIFO for o_proj).
            with tc.tile_pool(name="attnP", bufs=1, space="PSUM") as attnP, \
                 tc.tile_pool(name="attnS", bufs=1) as attnS:
                ok_sb = attnS.tile([128, 32], BF16, name="ok_sb")
                nc.sync.dma_start(ok_sb[:], ones_k[:])
                ob_sb = attnS.tile([128, 128], BF16, name="ob_sb")
                nc.sync.dma_start(ob_sb[:], ones_b[:])
                mk_sb = attnS.tile([128, 128], F32, name="mk_sb")
                nc.sync.dma_start(mk_sb[:], mask128[:])

                # ---------------- phase 1: QKV projections + RoPE ----------
                with tc.tile_pool(name="wres", bufs=1) as wres, \
                     tc.tile_pool(name="xs", bufs=36) as xs, \
                     tc.tile_pool(name="rp", bufs=3) as rp, \
                     tc.tile_pool(name="p1P", bufs=1, space="PSUM") as p1P:
                    wtiles = {}
                    for wname, w_dr in (("q", wqT), ("k", wkT), ("v", wvT)):
                        for c in range(NCT):
                            wt = wres.tile([128, DH], BF16, name=f"w{wname}_{c}")
                            nc.sync.dma_start(wt[:], w_dr[c * 128:(c + 1) * 128, :])
                            wtiles[(wname, c)] = wt

                    for t in range(NT):
                        tok = t * 512
                        cos_t = rp.tile([128, 512], BF16, tag="cos_t", name=f"cos{t}")
                        nc.sync.dma_start(cos_t[:], cos2E[:, tok:tok + 512])
                        sin_t = rp.tile([128, 512], BF16, tag="sin_t", name=f"sin{t}")
                        nc.sync.dma_start(sin_t[:], sin2E[:, tok:tok + 512])
                        xt = []
                        for c in range(NCT):
                            xc = xs.tile([128, 512], BF16, tag="xt", name=f"x_{t}_{c}")
                            nc.sync.dma_start(xc[:], xT[c * 128:(c + 1) * 128, tok:tok + 512])
                            xt.append(xc)
                        # interleave q/k (rope) and v psum groups so psum
                        # drains overlap the next group's matmuls
                        groups = [("q", 0), ("k", 0), ("v", 0), ("q", 1), ("k", 1),
                                  ("v", 1), ("q", 2), ("k", 2), ("v", 2), ("q", 3),
                                  ("k", 3), ("v", 3)]
                        for wname, i in groups:
                            if wname == "v":
                                tt = i
                                psv = p1P.tile([128, 512], F32, tag="ps", bufs=2,
                                               name=f"psv_{t}_{tt}")
                                for c in range(NCT):
                                    nc.tensor.matmul(
                                        psv[:], xt[c][:, tt * 128:(tt + 1) * 128],
                                        wtiles[("v", c)][:],
                                        start=(c == 0), stop=(c == NCT - 1))
                                vsb = rp.tile([128, 512], BF16, tag="vsb",
                                              name=f"vsb{t}{tt}")
                                nc.scalar.activation(
                                    vsb[:], psv[:], mybir.ActivationFunctionType.Copy)
                                nc.sync.dma_start(
                                    vd[tok + tt * 128: tok + (tt + 1) * 128, :], vsb[:])
                                continue
                            dst = qTd if wname == "q" else kTd
                            ps = p1P.tile([128, 512], F32, tag="ps", bufs=2,
                                          name=f"ps_{t}_{wname}{i}")
                            for c in range(NCT):
                                nc.tensor.matmul(
                                    ps[:], wtiles[(wname, c)][:, i * 128:(i + 1) * 128],
                                    xt[c][:], start=(c == 0), stop=(c == NCT - 1))
                            qsb = rp.tile([128, 512], BF16, tag="qsb", name=f"qsb{t}{wname}{i}")
                            nc.scalar.activation(
                                qsb[:], ps[:], mybir.ActivationFunctionType.Copy)
                            # swap 64-row halves (x0 <-> x1) via sbuf-sbuf DMA
                            qs2 = rp.tile([128, 512], BF16, tag="qs2", name=f"qs2{t}{wname}{i}")
                            nc.sync.dma_start(qs2[0:64, :], qsb[64:128, :])
                            nc.sync.dma_start(qs2[64:128, :], qsb[0:64, :])
                            qc = rp.tile([128, 512], BF16, tag="qc", name=f"qc{t}{wname}{i}")
                            nc.vector.tensor_tensor(qc[:], qsb[:], cos_t[:], mybir.AluOpType.mult)
                            qr = rp.tile([128, 512], BF16, tag="qr", name=f"qr{t}{wname}{i}")
                            nc.vector.tensor_tensor(qr[:], qs2[:], sin_t[:], mybir.AluOpType.mult)
                            qfin = rp.tile([128, 512], BF16, tag="qfin", name=f"qf{t}{wname}{i}")
                            nc.vector.tensor_tensor(qfin[:], qc[:], qr[:], mybir.AluOpType.add)
                            nc.sync.dma_start(dst[i * 128:(i + 1) * 128, tok:tok + 512], qfin[:])

                # ---------------- phase 2: attention + AllGathers ----------
                def issue_head_loads(b, h):
                    qh = attnS.tile([128, S], BF16, tag="qh", bufs=2, name=f"qh{b}{h}")
                    kh = attnS.tile([128, S], BF16, tag="kh", bufs=2, name=f"kh{b}{h}")
                    vh = attnS.tile([128, 16 * 128], BF16, tag="vh", bufs=2, name=f"vh{b}{h}")
                    nc.sync.dma_start(qh[:], qTd[h * 128:(h + 1) * 128, b * S:(b + 1) * S])
                    nc.sync.dma_start(kh[:], kTd[h * 128:(h + 1) * 128, b * S:(b + 1) * S])
                    nc.sync.dma_start(
                        vh[:].rearrange("p (kt d) -> p kt d", kt=16),
                        vd.rearrange("(bb kt p) i -> bb p kt i", bb=B, p=128)[b, :, :, h * 128:(h + 1) * 128])
                    return qh, kh, vh

                heads = [(b, h) for b in range(B) for h in range(HL)]
                pend = {heads[0]: issue_head_loads(*heads[0])}

                def fire_ag(bh):
                    nc.gpsimd.collective_compute(
                        "AllGather", mybir.AluOpType.bypass,
                        replica_groups=[list(range(NC))],
                        ins=[agin[bh].opt()],
                        outs=[agout[bh].opt()])

                for idx, (b, h) in enumerate(heads):
                    if idx + 1 < len(heads):
                        pend[heads[idx + 1]] = issue_head_loads(*heads[idx + 1])
                    if idx > 0:
                        # previous head's AllGather fires only after the next
                        # head's loads were issued, so the loads win the DMA
                        # rings over the collective
                        fire_ag(heads[idx - 1])
                    qh, kh, vh = pend.pop((b, h))
                    if True:
                        for pair in range(2):
                            sumsP = attnP.tile([128, 512], F32, tag="sums", bufs=1,
                                               name=f"sums{b}{h}{pair}")
                            accs = []
                            for l in range(2):
                                jq = 2 * pair + l
                                nkt = 4 * (jq + 1)
                                acc = attnP.tile([128, 512], F32, tag="acc", bufs=2,
                                                 name=f"acc{b}{h}{jq}")
                                accs.append(acc)

                                def score_exp(kt):
                                    d = kt - 4 * jq
                                    coff = 128 * d if d >= 0 else 0
                                    pss = attnP.tile([128, 512], F32, tag="pss", bufs=3,
                                                     name=f"pss{b}{h}{jq}{kt}")
                                    nc.tensor.matmul(
                                        pss[:, coff:], kh[:, kt * 128:(kt + 1) * 128],
                                        qh[:, jq * 512 + coff:(jq + 1) * 512],
                                        start=True, stop=True)
                                    if d >= 0:
                                        # additive causal mask on the diagonal
                                        # 128x128 block (-1e9 below diagonal)
                                        nc.vector.tensor_tensor(
                                            pss[:, coff:coff + 128], pss[:, coff:coff + 128],
                                            mk_sb[:], mybir.AluOpType.add)
                                    ex = attnS.tile([128, 512], BF16, tag="ex", bufs=6,
                                                    name=f"ex{b}{h}{jq}{kt}")
                                    nc.scalar.activation(
                                        ex[:, coff:], pss[:, coff:],
                                        mybir.ActivationFunctionType.Exp, scale=SCALE)
                                    return ex, coff

                                def pv_sums(kt, ex, coff):
                                    nc.tensor.matmul(acc[:, coff:], vh[:, kt * 128:(kt + 1) * 128],
                                                     ex[:, coff:],
                                                     start=(kt == 0), stop=(kt == nkt - 1))
                                    nc.tensor.matmul(sumsP[32 * l:32 * l + 32, coff:],
                                                     ok_sb[:], ex[:, coff:],
                                                     start=(kt == 0), stop=(kt == nkt - 1))

                                # two-kt software pipeline: emit scores(kt+2)
                                # before PV/sums(kt) so mask+exp latency hides
                                fifo = []
                                for kt in range(nkt):
                                    fifo.append((kt, score_exp(kt)))
                                    if len(fifo) > 2:
                                        k0, (ex0, c0) = fifo.pop(0)
                                        pv_sums(k0, ex0, c0)
                                for k0, (ex0, c0) in fifo:
                                    pv_sums(k0, ex0, c0)
                            recf = attnS.tile([64, 512], F32, tag="recf", bufs=2,
                                              name=f"recf{b}{h}{pair}")
                            nc.vector.reciprocal_approx_fast(recf[:], sumsP[0:64, :])
                            recb = attnS.tile([64, 512], BF16, tag="recb", bufs=2,
                                              name=f"recb{b}{h}{pair}")
                            nc.scalar.activation(recb[:], recf[:],
                                                 mybir.ActivationFunctionType.Copy)
                            for l in range(2):
                                jq = 2 * pair + l
                                rb = attnP.tile([128, 512], F32, tag="pss", bufs=3,
                                                name=f"rb{b}{h}{jq}")
                                nc.tensor.matmul(rb[:], ob_sb[32 * l:32 * l + 1, :],
                                                 recb[32 * l:32 * l + 1, :],
                                                 start=True, stop=True)
                                rbs = attnS.tile([128, 512], BF16, tag="rbs", bufs=2,
                                                 name=f"rbs{b}{h}{jq}")
                                nc.scalar.activation(rbs[:], rb[:],
                                                     mybir.ActivationFunctionType.Copy)
                                att = attnS.tile([128, 512], BF16, tag="att", bufs=3,
                                                 name=f"att{b}{h}{jq}")
                                nc.vector.tensor_tensor(att[:], accs[l][:], rbs[:],
                                                        mybir.AluOpType.mult)
                                nc.sync.dma_start(
                                    agin[(b, h)][:, jq * 512:(jq + 1) * 512], att[:])
                    if idx == len(heads) - 1:
                        fire_ag((b, h))

                # ---------------- phase 3: o_proj ----------------
                with tc.tile_pool(name="ores", bufs=1) as ores, \
                     tc.tile_pool(name="och", bufs=5) as och, \
                     tc.tile_pool(name="oo", bufs=4) as oo, \
                     tc.tile_pool(name="opP", bufs=1, space="PSUM") as opP:
                    wo_sb = ores.tile([128, NCT * DH], BF16, name="wo_sb")
                    nc.sync.dma_start(
                        wo_sb[:].rearrange("p (c i) -> p c i", c=NCT),
                        woT.rearrange("(c p) i -> p c i", p=128))
                    for t in range(T // 128):
                        bb = 0 if t < 16 else 1
                        tl = t % 16
                        ch = och.tile([128, NCT * 128], BF16, tag="ch", name=f"ch{t}")
                        # chunk c = r*4 + hh of the global head dim: gather the
                        # four per-head AllGather outputs side by side
                        chv = ch[:].rearrange("p (r hh u) -> p r hh u", r=NC, hh=HL)
                        for hh in range(HL):
                            nc.sync.dma_start(
                                chv[:, :, hh, :],
                                agout[(bb, hh)].rearrange("(r p) t -> p r t", p=128)[:, :, tl * 128:(tl + 1) * 128])
                        pso = opP.tile([128, 512], F32, tag="pso", bufs=2, name=f"pso{t}")
                        for i in range(NCT):
                            nc.tensor.matmul(pso[:], ch[:, i * 128:(i + 1) * 128],
                                             wo_sb[:, i * DH:(i + 1) * DH],
                                             start=(i == 0), stop=(i == NCT - 1))
                        ot = oo.tile([128, 512], F32, tag="ot", name=f"ot{t}")
                        nc.scalar.activation(
                            ot[:], pso[:], mybir.ActivationFunctionType.Copy)
                        nc.sync.dma_start(out[t * 128:(t + 1) * 128, :], ot[:])

    nc.compile()
    return nc


def _host_prep(x, freqs_cos, freqs_sin, mask, wq, wk, wv, wo):
    xT = np.ascontiguousarray(x.reshape(T, D).T).astype(bf16)
    cos = np.asarray(freqs_cos, np.float32).T   # [64, S]
    sin = np.asarray(freqs_sin, np.float32).T
    cos2 = np.concatenate([cos, cos], axis=0)           # [128, S]
    sin2 = np.concatenate([-sin, sin], axis=0)          # sign-folded
    cos2E = np.tile(cos2, (1, B)).astype(bf16)          # [128, T] b-major
    sin2E = np.tile(sin2, (1, B)).astype(bf16)
    # head-dim permutation: evens then odds within each 128-row head block
    perm = np.arange(D).reshape(H, HD // 2, 2).transpose(0, 2, 1).reshape(D)
    ones_k = np.ones((128, 32), bf16)
    ones_b = np.ones((128, 128), bf16)
    # rows = keys, cols = queries: mask key>query = strictly lower triangle
    mask128 = np.tril(np.full((128, 128), -1e9, np.float32), k=-1)
    shared = dict(xT=xT, cos2E=cos2E, sin2E=sin2E, ones_k=ones_k, ones_b=ones_b,
                  mask128=mask128)
    wq_p = np.asarray(wq, np.float32)[perm, :]
    wk_p = np.asarray(wk, np.float32)[perm, :]
    in_maps = []
    for r in range(NC):
        sl = slice(r * DH, (r + 1) * DH)
        m = dict(shared)
        m["wqT"] = np.ascontiguousarray(wq_p[sl, :].T).astype(bf16)
        m["wkT"] = np.ascontiguousarray(wk_p[sl, :].T).astype(bf16)
        m["wvT"] = np.ascontiguousarray(np.asarray(wv, np.float32)[sl, :].T).astype(bf16)
        m["woT"] = np.ascontiguousarray(np.asarray(wo, np.float32)[sl, :].T).astype(bf16)
        in_maps.append(m)
    return in_maps


def kernel(x, freqs_cos, freqs_sin, mask, wq, wk, wv, wo, start_pos):
    global LAST_RESULT
    if "nc" not in _CACHE:
        _CACHE["nc"] = build()
    nc = _CACHE["nc"]
    in_maps = _host_prep(x, freqs_cos, freqs_sin, mask, wq, wk, wv, wo)
    res = run_bass_kernel_spmd(nc, in_maps, core_ids=list(range(NC)))
    LAST_RESULT = res
    parts = [res.results[r]["out"] for r in range(NC)]
    full = np.concatenate(parts, axis=1)      # [T, D]
    return np.ascontiguousarray(full.reshape(B, S, D)).astype(np.float32)


# revision 27
# speedup vs baseline: 1.0297x; 1.0297x over previous
"""Trainium2 8-core tensor-parallel attention kernel (Bass/Tile), v2.

Full inputs in, full output out. Sharding: tensor-parallel over heads
(4 heads per core), per-batch AllGather of attention outputs overlapped
with compute, each core computes a 512-wide output-column slice of the
o_proj; host concatenates.

v2 vs baseline:
- RoPE via evens/odds head-dim permutation (folded into wq/wk host-side):
  the pair rotation becomes two contiguous 64-partition SBUF-SBUF DMA
  copies + sign-folded sin, removing the rotation matmul and a PSUM bank.
- Causal trim: score/PV/sum matmuls skip fully-masked 128-column blocks.
- Mask via affine_select on the exp output (no mask tensor at all).
- Row-sums batched into a shared PSUM tile (32-row replicated ones), one
  fast reciprocal per jq-pair.
- PSUM/SBUF pools nested so attention overlaps phase 1 (b1) and o_proj
  (b0) overlaps attention (b1); AllGathers hide under compute.
"""
import sys

for _p in ("/opt/trn_rl_repo",):
    if _p not in sys.path:
        sys.path.insert(0, _p)

import numpy as np
import ml_dtypes

import concourse.bass as bass
import concourse.mybir as mybir
import concourse.tile as tile
from concourse import bacc
from concourse.bass_utils import run_bass_kernel_spmd

B, S, D, H = 2, 2048, 4096, 32
HD = D // H          # 128 head dim
T = B * S            # 4096 tokens
NC = 8               # cores
HL = H // NC         # 4 heads per core
DH = HL * HD         # 512 dims per core
SCALE = 1.0 / float(np.sqrt(HD))
BF16 = mybir.dt.bfloat16
F32 = mybir.dt.float32
bf16 = ml_dtypes.bfloat16

_CACHE = {}
LAST_RESULT = None


def build():
    nc = bacc.Bacc("TRN2", target_bir_lowering=False, debug=False, num_devices=NC)

    xT = nc.dram_tensor("xT", [D, T], BF16, kind="ExternalInput").ap()
    wqT = nc.dram_tensor("wqT", [D, DH], BF16, kind="ExternalInput").ap()
    wkT = nc.dram_tensor("wkT", [D, DH], BF16, kind="ExternalInput").ap()
    wvT = nc.dram_tensor("wvT", [D, DH], BF16, kind="ExternalInput").ap()
    woT = nc.dram_tensor("woT", [D, DH], BF16, kind="ExternalInput").ap()
    cos2E = nc.dram_tensor("cos2E", [HD, T], BF16, kind="ExternalInput").ap()
    sin2E = nc.dram_tensor("sin2E", [HD, T], BF16, kind="ExternalInput").ap()
    ones_k = nc.dram_tensor("ones_k", [128, 32], BF16, kind="ExternalInput").ap()
    ones_b = nc.dram_tensor("ones_b", [128, 128], BF16, kind="ExternalInput").ap()
    mask128 = nc.dram_tensor("mask128", [128, 128], F32, kind="ExternalInput").ap()
    out = nc.dram_tensor("out", [T, DH], F32, kind="ExternalOutput").ap()

    NT = T // 512      # 8 token slices of 512
    NCT = D // 128     # 32 contraction tiles

    with tile.TileContext(nc) as tc:
        with tc.tile_pool(name="dram", bufs=1, space="DRAM") as dram:
            qTd = dram.tile([DH, T], BF16)
            kTd = dram.tile([DH, T], BF16)
            vd = dram.tile([T, DH], BF16)
            agin = {(b, h): dram.tile([128, S], BF16, name=f"agin{b}{h}")
                    for b in range(B) for h in range(HL)}
            agout = {(b, h): dram.tile([NC * 128, S], BF16, addr_space="Shared",
                                       name=f"agout{b}{h}")
                     for b in range(B) for h in range(HL)}

            # Outer pools spanning attention + o_proj (opened first so the
            # phase-1 pools below release their space LIFO for o_proj).
            with tc.tile_pool(name="attnP", bufs=1, space="PSUM") as attnP, \
                 tc.tile_pool(name="attnS", bufs=1) as attnS:
                ok_sb = attnS.tile([128, 32], BF16, name="ok_sb")
                nc.sync.dma_start(ok_sb[:], ones_k[:])
                ob_sb = attnS.tile([128, 128], BF16, name="ob_sb")
                nc.sync.dma_start(ob_sb[:], ones_b[:])
                mk_sb = attnS.tile([128, 128], F32, name="mk_sb")
                nc.sync.dma_start(mk_sb[:], mask128[:])

                # ---------------- phase 1: QKV projections + RoPE ----------
                with tc.tile_pool(name="wres", bufs=1) as wres, \
                     tc.tile_pool(name="xs", bufs=36) as xs, \
                     tc.tile_pool(name="rp", bufs=3) as rp, \
                     tc.tile_pool(name="p1P", bufs=1, space="PSUM") as p1P:
                    wtiles = {}
                    for wname, w_dr in (("q", wqT), ("k", wkT), ("v", wvT)):
                        for c in range(NCT):
                            wt = wres.tile([128, DH], BF16, name=f"w{wname}_{c}")
                            nc.sync.dma_start(wt[:], w_dr[c * 128:(c + 1) * 128, :])
                            wtiles[(wname, c)] = wt

                    for t in range(NT):
                        tok = t * 512
                        cos_t = rp.tile([128, 512], BF16, tag="cos_t", name=f"cos{t}")
                        nc.sync.dma_start(cos_t[:], cos2E[:, tok:tok + 512])
                        sin_t = rp.tile([128, 512], BF16, tag="sin_t", name=f"sin{t}")
                        nc.sync.dma_start(sin_t[:], sin2E[:, tok:tok + 512])
                        xt = []
                        for c in range(NCT):
                            xc = xs.tile([128, 512], BF16, tag="xt", name=f"x_{t}_{c}")
                            nc.sync.dma_start(xc[:], xT[c * 128:(c + 1) * 128, tok:tok + 512])
                            xt.append(xc)
                        # interleave q/k (rope) and v psum groups so psum
                        # drains overlap the next group's matmuls
                        groups = [("q", 0), ("k", 0), ("v", 0), ("q", 1), ("k", 1),
                                  ("v", 1), ("q", 2), ("k", 2), ("v", 2), ("q", 3),
                                  ("k", 3), ("v", 3)]
                        for wname, i in groups:
                            if wname == "v":
                                tt = i
                                psv = p1P.tile([128, 512], F32, tag="ps", bufs=2,
                                               name=f"psv_{t}_{tt}")
                                for c in range(NCT):
                                    nc.tensor.matmul(
                                        psv[:], xt[c][:, tt * 128:(tt + 1) * 128],
                                        wtiles[("v", c)][:],
                                        start=(c == 0), stop=(c == NCT - 1))
                                vsb = rp.tile([128, 512], BF16, tag="vsb",
                                              name=f"vsb{t}{tt}")
                                nc.scalar.activation(
                                    vsb[:], psv[:], mybir.ActivationFunctionType.Copy)
                                nc.sync.dma_start(
                                    vd[tok + tt * 128: tok + (tt + 1) * 128, :], vsb[:])
                                continue
                            dst = qTd if wname == "q" else kTd
                            ps = p1P.tile([128, 512], F32, tag="ps", bufs=2,
                                          name=f"ps_{t}_{wname}{i}")
                            for c in range(NCT):
                                nc.tensor.matmul(
                                    ps[:], wtiles[(wname, c)][:, i * 128:(i + 1) * 128],
                                    xt[c][:], start=(c == 0), stop=(c == NCT - 1))
                            qsb = rp.tile([128, 512], BF16, tag="qsb", name=f"qsb{t}{wname}{i}")
                            nc.scalar.activation(
                                qsb[:], ps[:], mybir.ActivationFunctionType.Copy)
                            # swap 64-row halves (x0 <-> x1) via sbuf-sbuf DMA
                            qs2 = rp.tile([128, 512], BF16, tag="qs2", name=f"qs2{t}{wname}{i}")
                            nc.sync.dma_start(qs2[0:64, :], qsb[64:128, :])
                            nc.sync.dma_start(qs2[64:128, :], qsb[0:64, :])
                            qc = rp.tile([128, 512], BF16, tag="qc", name=f"qc{t}{wname}{i}")
                            nc.vector.tensor_tensor(qc[:], qsb[:], cos_t[:], mybir.AluOpType.mult)
                            qr = rp.tile([128, 512], BF16, tag="qr", name=f"qr{t}{wname}{i}")
                            nc.vector.tensor_tensor(qr[:], qs2[:], sin_t[:], mybir.AluOpType.mult)
                            qfin = rp.tile([128, 512], BF16, tag="qfin", name=f"qf{t}{wname}{i}")
                            nc.vector.tensor_tensor(qfin[:], qc[:], qr[:], mybir.AluOpType.add)
                            nc.sync.dma_start(dst[i * 128:(i + 1) * 128, tok:tok + 512], qfin[:])

                # ---------------- phase 2: attention + AllGathers ----------
                def issue_head_loads(b, h):
                    qh = attnS.tile([128, S], BF16, tag="qh", bufs=2, name=f"qh{b}{h}")
                    kh = attnS.tile([128, S], BF16, tag="kh", bufs=2, name=f"kh{b}{h}")
                    vh = attnS.tile([128, 16 * 128], BF16, tag="vh", bufs=2, name=f"vh{b}{h}")
                    nc.sync.dma_start(qh[:], qTd[h * 128:(h + 1) * 128, b * S:(b + 1) * S])
                    nc.sync.dma_start(kh[:], kTd[h * 128:(h + 1) * 128, b * S:(b + 1) * S])
                    nc.sync.dma_start(
                        vh[:].rearrange("p (kt d) -> p kt d", kt=16),
                        vd.rearrange("(bb kt p) i -> bb p kt i", bb=B, p=128)[b, :, :, h * 128:(h + 1) * 128])
                    return qh, kh, vh

                heads = [(b, h) for b in range(B) for h in range(HL)]
                pend = {heads[0]: issue_head_loads(*heads[0])}

                def fire_ag(bh):
                    nc.gpsimd.collective_compute(
                        "AllGather", mybir.AluOpType.bypass,
                        replica_groups=[list(range(NC))],
                        ins=[agin[bh].opt()],
                        outs=[agout[bh].opt()])

                for idx, (b, h) in enumerate(heads):
                    if idx + 1 < len(heads):
                        pend[heads[idx + 1]] = issue_head_loads(*heads[idx + 1])
                    qh, kh, vh = pend.pop((b, h))
                    if True:
                        for pair in range(2):
                            sumsP = attnP.tile([128, 512], F32, tag="sums", bufs=1,
                                               name=f"sums{b}{h}{pair}")
                            accs = []
                            for l in range(2):
                                jq = 2 * pair + l
                                nkt = 4 * (jq + 1)
                                acc = attnP.tile([128, 512], F32, tag="acc", bufs=2,
                                                 name=f"acc{b}{h}{jq}")
                                accs.append(acc)

                                def score_exp(kt):
                                    d = kt - 4 * jq
                                    coff = 128 * d if d >= 0 else 0
                                    pss = attnP.tile([128, 512], F32, tag="pss", bufs=3,
                                                     name=f"pss{b}{h}{jq}{kt}")
                                    nc.tensor.matmul(
                                        pss[:, coff:], kh[:, kt * 128:(kt + 1) * 128],
                                        qh[:, jq * 512 + coff:(jq + 1) * 512],
                                        start=True, stop=True)
                                    if d >= 0:
                                        # additive causal mask on the diagonal
                                        # 128x128 block (-1e9 below diagonal)
                                        nc.vector.tensor_tensor(
                                            pss[:, coff:coff + 128], pss[:, coff:coff + 128],
                                            mk_sb[:], mybir.AluOpType.add)
                                    ex = attnS.tile([128, 512], BF16, tag="ex", bufs=6,
                                                    name=f"ex{b}{h}{jq}{kt}")
                                    nc.scalar.activation(
                                        ex[:, coff:], pss[:, coff:],
                                        mybir.ActivationFunctionType.Exp, scale=SCALE)
                                    return ex, coff

                                def pv_sums(kt, ex, coff):
                                    nc.tensor.matmul(acc[:, coff:], vh[:, kt * 128:(kt + 1) * 128],
                                                     ex[:, coff:],
                                                     start=(kt == 0), stop=(kt == nkt - 1))
                                    nc.tensor.matmul(sumsP[32 * l:32 * l + 32, coff:],
                                                     ok_sb[:], ex[:, coff:],
                                                     start=(kt == 0), stop=(kt == nkt - 1))

                                # two-kt software pipeline: emit scores(kt+2)
                                # before PV/sums(kt) so mask+exp latency hides
                                fifo = []
                                for kt in range(nkt):
                                    fifo.append((kt, score_exp(kt)))
                                    if len(fifo) > 2:
                                        k0, (ex0, c0) = fifo.pop(0)
                                        pv_sums(k0, ex0, c0)
                                for k0, (ex0, c0) in fifo:
                                    pv_sums(k0, ex0, c0)
                            recf = attnS.tile([64, 512], F32, tag="recf", bufs=2,
                                              name=f"recf{b}{h}{pair}")
                            nc.vector.reciprocal_approx_fast(recf[:], sumsP[0:64, :])
                            recb = attnS.tile([64, 512], BF16, tag="recb", bufs=2,
                                              name=f"recb{b}{h}{pair}")
                            nc.scalar.activation(recb[:], recf[:],
                                                 mybir.ActivationFunctionType.Copy)
                            for l in range(2):
                                jq = 2 * pair + l
                                rb = attnP.tile([128, 512], F32, tag="pss", bufs=3,
                                                name=f"rb{b}{h}{jq}")
                                nc.tensor.matmul(rb[:], ob_sb[32 * l:32 * l + 1, :],
                                                 recb[32 * l:32 * l + 1, :],
                                                 start=True, stop=True)
                                rbs = attnS.tile([128, 512], BF16, tag="rbs", bufs=2,
                                                 name=f"rbs{b}{h}{jq}")
                                nc.scalar.activation(rbs[:], rb[:],
                                                     mybir.ActivationFunctionType.Copy)
                                att = attnS.tile([128, 512], BF16, tag="att", bufs=3,
                                                 name=f"att{b}{h}{jq}")
                                nc.vector.tensor_tensor(att[:], accs[l][:], rbs[:],
                                                        mybir.AluOpType.mult)
                                nc.sync.dma_start(
                                    agin[(b, h)][:, jq * 512:(jq + 1) * 512], att[:])
                    # fire this head's AllGather immediately so link traffic
                    # spreads under the remaining attention compute
                    fire_ag((b, h))

                # ---------------- phase 3: o_proj ----------------
                with tc.tile_pool(name="ores", bufs=1) as ores, \
                     tc.tile_pool(name="och", bufs=5) as och, \
                     tc.tile_pool(name="oo", bufs=4) as oo, \
                     tc.tile_pool(name="opP", bufs=1, space="PSUM") as opP:
                    wo_sb = ores.tile([128, NCT * DH], BF16, name="wo_sb")
                    nc.sync.dma_start(
                        wo_sb[:].rearrange("p (c i) -> p c i", c=NCT),
                        woT.rearrange("(c p) i -> p c i", p=128))
                    for t in range(T // 128):
                        bb = 0 if t < 16 else 1
                        tl = t % 16
                        ch = och.tile([128, NCT * 128], BF16, tag="ch", name=f"ch{t}")
                        # chunk c = r*4 + hh of the global head dim: gather the
                        # four per-head AllGather outputs side by side
                        chv = ch[:].rearrange("p (r hh u) -> p r hh u", r=NC, hh=HL)
                        for hh in range(HL):
                            nc.sync.dma_start(
                                chv[:, :, hh, :],
                                agout[(bb, hh)].rearrange("(r p) t -> p r t", p=128)[:, :, tl * 128:(tl + 1) * 128])
                        pso = opP.tile([128, 512], F32, tag="pso", bufs=2, name=f"pso{t}")
                        for i in range(NCT):
                            nc.tensor.matmul(pso[:], ch[:, i * 128:(i + 1) * 128],
                                             wo_sb[:, i * DH:(i + 1) * DH],
                                             start=(i == 0), stop=(i == NCT - 1))
                        ot = oo.tile([128, 512], F32, tag="ot", name=f"ot{t}")
                        nc.scalar.activation(
                            ot[:], pso[:], mybir.ActivationFunctionType.Copy)
                        nc.sync.dma_start(out[t * 128:(t + 1) * 128, :], ot[:])

    nc.compile()
    return nc


def _host_prep(x, freqs_cos, freqs_sin, mask, wq, wk, wv, wo):
    xT = np.ascontiguousarray(x.reshape(T, D).T).astype(bf16)
    cos = np.asarray(freqs_cos, np.float32).T   # [64, S]
    sin = np.asarray(freqs_sin, np.float32).T
    cos2 = np.concatenate([cos, cos], axis=0)           # [128, S]
    sin2 = np.concatenate([-sin, sin], axis=0)          # sign-folded
    cos2E = np.tile(cos2, (1, B)).astype(bf16)          # [128, T] b-major
    sin2E = np.tile(sin2, (1, B)).astype(bf16)
    # head-dim permutation: evens then odds within each 128-row head block
    perm = np.arange(D).reshape(H, HD // 2, 2).transpose(0, 2, 1).reshape(D)
    ones_k = np.ones((128, 32), bf16)
    ones_b = np.ones((128, 128), bf16)
    # rows = keys, cols = queries: mask key>query = strictly lower triangle
    mask128 = np.tril(np.full((128, 128), -1e9, np.float32), k=-1)
    shared = dict(xT=xT, cos2E=cos2E, sin2E=sin2E, ones_k=ones_k, ones_b=ones_b,
                  mask128=mask128)
    wq_p = np.asarray(wq, np.float32)[perm, :]
    wk_p = np.asarray(wk, np.float32)[perm, :]
    in_maps = []
    for r in range(NC):
        sl = slice(r * DH, (r + 1) * DH)
        m = dict(shared)
        m["wqT"] = np.ascontiguousarray(wq_p[sl, :].T).astype(bf16)
        m["wkT"] = np.ascontiguousarray(wk_p[sl, :].T).astype(bf16)
        m["wvT"] = np.ascontiguousarray(np.asarray(wv, np.float32)[sl, :].T).astype(bf16)
        m["woT"] = np.ascontiguousarray(np.asarray(wo, np.float32)[sl, :].T).astype(bf16)
        in_maps.append(m)
    return in_maps


def kernel(x, freqs_cos, freqs_sin, mask, wq, wk, wv, wo, start_pos):
    global LAST_RESULT
    if "nc" not in _CACHE:
        _CACHE["nc"] = build()
    nc = _CACHE["nc"]
    in_maps = _host_prep(x, freqs_cos, freqs_sin, mask, wq, wk, wv, wo)
    res = run_bass_kernel_spmd(nc, in_maps, core_ids=list(range(NC)))
    LAST_RESULT = res
    parts = [res.results[r]["out"] for r in range(NC)]
    full = np.concatenate(parts, axis=1)      # [T, D]
    return np.ascontiguousarray(full.reshape(B, S, D)).astype(np.float32)
